# revision 16
# baseline (speedup 1.0000x reference)
"""DifferentiableLogicLayer Trainium2 kernel.

Math: reference computes, per batch row t and gate g (G = INPUT_SIZE = 8192):
    a = x[t, g], b = x[t, (g+1) % 8192]            (x uniform in [0,1] -> clip no-op)
    out[t, g] = sum_o softmax(gate_logits[g])_o * op_o(a, b)
Each of the 16 soft ops is linear in {1, a, b, ab}, so with probs p:
    out = C0 + CA*a + CB*b + CAB*a*b
    C0  = p8+..+p15
    CA  = p2+p3+p6+p7-p8-p9-p12-p13
    CB  = p4+p5+p6+p7-p8-p9-p10-p11
    CAB = p1-p2-p4-2*p6-p7+p8+2*p9+p11+p13-p14
Factored: out = ((CAB*a + CB)*b) + (CA*a + C0)  -> 6 elementwise passes.

Sharding: gates across the 8 cores (1024 each). Per-core inputs:
    xs  [2048, 1025] = x cols [1024c .. 1024c+1024] (halo col, wraparound)
    glT [16, 1024]   = gate_logits rows for this core's gates, transposed
    wsel [16, 5]     = constant selector: columns = (den, N0, NA, NB, NAB)
                       weights over the 16 exp'd logits

Coefficient prep: exp(glT) on ScalarE -> one K=16 matmul with wsel gives all 5
numerator rows [5, G] in PSUM -> reciprocal + 4 row multiplies on VectorE give
C* rows [1, G] -> K=1 matmuls (ones x row) broadcast each to a [128, G] PSUM
tile.  The numerator strip shares R_c0's PSUM banks (R_c0 is broadcast last).

Engine assignment (measured port-sharing rule: GPSIMD's SBUF port is
VectorE's rd1, so GP only contends with DVE instructions whose BOTH tensor
operands live in SBUF):
    VectorE: u = a*R_cab, u += R_cb, v = a*R_ca, v += R_c0   (rd0 + PSUM)
    GPSIMD:  w = u*b, o = w + v                              (pure SBUF)
VectorE runs MEGA=2 batch tiles per instruction (3D APs, step-0 broadcast on
the coefficient operand); GPSIMD keeps flat 2D per-subtile APs (3D APs are
~20% slower on the Q7s). First/last groups run at mega=1 to shorten the
pipeline ramp and tail.
"""

import numpy as np

NUM_GATES = 8192
INPUT_SIZE = 8192
BATCH = 2048
N_CORES = 8
G = NUM_GATES // N_CORES  # 1024 local gates
P = 128
MEGA = 2

# wsel [16, 128]: numerator selector columns placed so the K=16 matmul lands
# each row on a quadrant-aligned partition (PSUM reads must be 32-aligned):
#   col 0 = den, col 32 = N0, col 64 = NA, col 96 = NB, col 1 = NAB
# (col 1 is consumed by a separate M=1 matmul so NAB lands on partition 0.)
_WSEL = np.zeros((16, 128), dtype=np.float32)
_WSEL[:, 0] = 1.0
_WSEL[8:16, 32] = 1.0
for _o in (2, 3, 6, 7):
    _WSEL[_o, 64] = 1.0
for _o in (8, 9, 12, 13):
    _WSEL[_o, 64] = -1.0
for _o in (4, 5, 6, 7):
    _WSEL[_o, 96] = 1.0
for _o in (8, 9, 10, 11):
    _WSEL[_o, 96] = -1.0
for _o, _w in ((1, 1), (2, -1), (4, -1), (6, -2), (7, -1), (8, 1), (9, 2),
               (11, 1), (13, 1), (14, -1)):
    _WSEL[_o, 1] = float(_w)

_CACHE = {}


def _build_nc(reps=1, mega=MEGA):
    from contextlib import ExitStack

    import concourse.bacc as bacc
    import concourse.mybir as mybir
    from concourse.mybir import AluOpType as Op
    from concourse.tile import TileContext

    f32 = mybir.dt.float32
    Act = mybir.ActivationFunctionType

    nc = bacc.Bacc("TRN2", target_bir_lowering=False, debug=False,
                   num_devices=N_CORES)
    xs = nc.dram_tensor("xs", [BATCH, G + 1], f32, kind="ExternalInput").ap()
    glT = nc.dram_tensor("glT", [16, G], f32, kind="ExternalInput").ap()
    wsel = nc.dram_tensor("wsel", [16, P], f32, kind="ExternalInput").ap()
    out = nc.dram_tensor("out", [BATCH, G], f32, kind="ExternalOutput").ap()

    with TileContext(nc) as tc, ExitStack() as ctx:
        cpool = ctx.enter_context(tc.tile_pool(name="coef", bufs=1))
        ppool = ctx.enter_context(tc.tile_pool(name="psum", bufs=1, space="PSUM"))
        xpool = ctx.enter_context(tc.tile_pool(name="x", bufs=4))
        upool = ctx.enter_context(tc.tile_pool(name="tu", bufs=4))
        vpool = ctx.enter_context(tc.tile_pool(name="tv", bufs=4))
        wpool = ctx.enter_context(tc.tile_pool(name="tw", bufs=3))
        opool = ctx.enter_context(tc.tile_pool(name="o", bufs=3))

        for rep in range(reps):
            # ---- coefficient prep ----
            lgT = cpool.tile([16, G], f32, name=f"lgT{rep}")
            nc.scalar.dma_start(out=lgT[:, :], in_=glT[:, :])
            wt = cpool.tile([16, P], f32, name=f"wt{rep}")
            nc.scalar.dma_start(out=wt[:, :], in_=wsel[:, :])
            ones = cpool.tile([1, P], f32, name=f"ones{rep}")
            nc.vector.memset(ones[:, :], 1.0)

            R = {nm: ppool.tile([P, G], f32, name=f"R_{nm}{rep}")
                 for nm in ("cab", "cb", "ca", "c0")}

            # warm the PE while the coefficient chain runs
            nc.tensor.matmul(R["cab"][:, 0:P], ones[:, :], ones[:, :],
                             start=True, stop=True)

            ET = cpool.tile([16, G], f32, name=f"ET{rep}")
            nc.scalar.activation(ET[:, :], lgT[:, :], Act.Exp)

            # numerator rows: den@p0, N0@p32, NA@p64, NB@p96 of R_c0's banks
            # (freed by broadcasting c0 last); NAB@p0 of R_cab's banks.
            num = R["c0"][:, :]
            nab = R["cab"][0:1, :]
            for j in range(0, G, 512):
                nc.tensor.matmul(num[:, j:j + 512], wt[:, :], ET[:, j:j + 512],
                                 start=True, stop=True)
                nc.tensor.matmul(nab[:, j:j + 512], wt[:, 1:2], ET[:, j:j + 512],
                                 start=True, stop=True)

            rden = cpool.tile([1, G], f32, name=f"rden{rep}")
            nc.vector.reciprocal(rden[:, :], num[0:1, :])

            # rows (order matters: cab first; c0 last frees the num strip)
            for nm, src in (("cab", nab), ("cb", num[96:97, :]),
                            ("ca", num[64:65, :]), ("c0", num[32:33, :])):
                row = cpool.tile([1, G], f32, name=f"row_{nm}{rep}")
                nc.vector.tensor_tensor(row[:, :], src, rden[:, :], Op.mult)
                for j in range(0, G, 512):
                    nc.tensor.matmul(R[nm][:, j:j + 512], ones[:, :],
                                     row[:, j:j + 512], start=True, stop=True)

            def bc(r, m):
                return r[:, :].unsqueeze(1).broadcast_to([P, m, G])

            # ---- main loop; group sizes in batch tiles (first/last small) --
            sizes = [1, 1] + [mega] * ((BATCH // P - 4) // mega) + [1, 1]
            assert sum(sizes) == BATCH // P
            rows_lo = 0
            for gi, m in enumerate(sizes):
                rp = P * m
                xin = xs[rows_lo:rows_lo + rp, :].rearrange("(m p) c -> p m c", m=m)
                xt = xpool.tile([P, m, G + 1], f32, name=f"xt{rep}_{gi}",
                                tag="xt")
                nc.sync.dma_start(out=xt[:, :, :], in_=xin)
                a = xt[:, :, 0:G]

                u = upool.tile([P, m, G], f32, name=f"u{rep}_{gi}", tag="u")
                v = vpool.tile([P, m, G], f32, name=f"v{rep}_{gi}", tag="v")
                nc.vector.tensor_tensor(u[:, :, :], a, bc(R["cab"], m), Op.mult)
                nc.vector.tensor_tensor(u[:, :, :], u[:, :, :], bc(R["cb"], m), Op.add)
                nc.vector.tensor_tensor(v[:, :, :], a, bc(R["ca"], m), Op.mult)
                nc.vector.tensor_tensor(v[:, :, :], v[:, :, :], bc(R["c0"], m), Op.add)

                w = wpool.tile([P, m, G], f32, name=f"w{rep}_{gi}", tag="w")
                o = opool.tile([P, m, G], f32, name=f"o{rep}_{gi}", tag="o")
                for sm in range(m):
                    nc.gpsimd.tensor_tensor(w[:, sm, :], u[:, sm, :],
                                            xt[:, sm, 1:G + 1], Op.mult)
                    nc.gpsimd.tensor_tensor(o[:, sm, :], w[:, sm, :],
                                            v[:, sm, :], Op.add)
                    nc.sync.dma_start(
                        out=out[rows_lo + sm * P:rows_lo + (sm + 1) * P, :],
                        in_=o[:, sm, :])
                rows_lo += rp

    nc.compile()
    return nc


def _get_nc(reps=1, mega=MEGA):
    key = (reps, mega)
    if key not in _CACHE:
        _CACHE[key] = _build_nc(reps, mega)
    return _CACHE[key]


def _shard_inputs(x, gate_logits):
    x = np.ascontiguousarray(x, dtype=np.float32)
    gate_logits = np.ascontiguousarray(gate_logits, dtype=np.float32)
    xs_full = np.concatenate([x, x[:, :1]], axis=1)  # wraparound halo
    in_maps = []
    for c in range(N_CORES):
        in_maps.append({
            "xs": np.ascontiguousarray(xs_full[:, c * G:c * G + G + 1]),
            "glT": np.ascontiguousarray(gate_logits[c * G:(c + 1) * G].T),
            "wsel": _WSEL,
        })
    return in_maps


def kernel(x, gate_logits):
    from concourse.bass_utils import run_bass_kernel_spmd

    nc = _get_nc()
    in_maps = _shard_inputs(x, gate_logits)
    res = run_bass_kernel_spmd(nc, in_maps, core_ids=list(range(N_CORES)))
    return np.concatenate([res.results[c]["out"] for c in range(N_CORES)], axis=1)


# revision 18
# speedup vs baseline: 1.0373x; 1.0373x over previous
"""DifferentiableLogicLayer Trainium2 kernel.

Math: reference computes, per batch row t and gate g (G = INPUT_SIZE = 8192):
    a = x[t, g], b = x[t, (g+1) % 8192]            (x uniform in [0,1] -> clip no-op)
    out[t, g] = sum_o softmax(gate_logits[g])_o * op_o(a, b)
Each of the 16 soft ops is linear in {1, a, b, ab}, so with probs p:
    out = C0 + CA*a + CB*b + CAB*a*b
    C0  = p8+..+p15
    CA  = p2+p3+p6+p7-p8-p9-p12-p13
    CB  = p4+p5+p6+p7-p8-p9-p10-p11
    CAB = p1-p2-p4-2*p6-p7+p8+2*p9+p11+p13-p14
Factored: out = ((CAB*a + CB)*b) + (CA*a + C0)  -> 6 elementwise passes.

Sharding: gates across the 8 cores (1024 each). Per-core inputs:
    xs  [2048, 1025] = x cols [1024c .. 1024c+1024] (halo col, wraparound)
    glT [16, 1024]   = gate_logits rows for this core's gates, transposed
    wsel [16, 5]     = constant selector: columns = (den, N0, NA, NB, NAB)
                       weights over the 16 exp'd logits

Coefficient prep: exp(glT) on ScalarE -> one K=16 matmul with wsel gives all 5
numerator rows [5, G] in PSUM -> reciprocal + 4 row multiplies on VectorE give
C* rows [1, G] -> K=1 matmuls (ones x row) broadcast each to a [128, G] PSUM
tile.  The numerator strip shares R_c0's PSUM banks (R_c0 is broadcast last).

Engine assignment (measured port-sharing rule: GPSIMD's SBUF port is
VectorE's rd1, so GP only contends with DVE instructions whose BOTH tensor
operands live in SBUF):
    VectorE: u = a*R_cab, u += R_cb, v = a*R_ca, v += R_c0   (rd0 + PSUM)
    GPSIMD:  w = u*b, o = w + v                              (pure SBUF)
VectorE runs MEGA=2 batch tiles per instruction (3D APs, step-0 broadcast on
the coefficient operand); GPSIMD keeps flat 2D per-subtile APs (3D APs are
~20% slower on the Q7s). First/last groups run at mega=1 to shorten the
pipeline ramp and tail.
"""

import numpy as np

NUM_GATES = 8192
INPUT_SIZE = 8192
BATCH = 2048
N_CORES = 8
G = NUM_GATES // N_CORES  # 1024 local gates
P = 128
MEGA = 2

# wsel [16, 128]: numerator selector columns placed so the K=16 matmul lands
# each row on a quadrant-aligned partition (PSUM reads must be 32-aligned):
#   col 0 = den, col 32 = N0, col 64 = NA, col 96 = NB, col 1 = NAB
# (col 1 is consumed by a separate M=1 matmul so NAB lands on partition 0.)
_WSEL = np.zeros((16, 128), dtype=np.float32)
_WSEL[:, 0] = 1.0
_WSEL[8:16, 32] = 1.0
for _o in (2, 3, 6, 7):
    _WSEL[_o, 64] = 1.0
for _o in (8, 9, 12, 13):
    _WSEL[_o, 64] = -1.0
for _o in (4, 5, 6, 7):
    _WSEL[_o, 96] = 1.0
for _o in (8, 9, 10, 11):
    _WSEL[_o, 96] = -1.0
for _o, _w in ((1, 1), (2, -1), (4, -1), (6, -2), (7, -1), (8, 1), (9, 2),
               (11, 1), (13, 1), (14, -1)):
    _WSEL[_o, 1] = float(_w)

_CACHE = {}


def _build_nc(reps=1, mega=MEGA):
    from contextlib import ExitStack

    import concourse.bacc as bacc
    import concourse.mybir as mybir
    from concourse.mybir import AluOpType as Op
    from concourse.tile import TileContext

    f32 = mybir.dt.float32
    Act = mybir.ActivationFunctionType

    nc = bacc.Bacc("TRN2", target_bir_lowering=False, debug=False,
                   num_devices=N_CORES)
    xs = nc.dram_tensor("xs", [BATCH, G + 1], f32, kind="ExternalInput").ap()
    glT = nc.dram_tensor("glT", [16, G], f32, kind="ExternalInput").ap()
    wsel = nc.dram_tensor("wsel", [16, P], f32, kind="ExternalInput").ap()
    out = nc.dram_tensor("out", [BATCH, G], f32, kind="ExternalOutput").ap()

    with TileContext(nc) as tc, ExitStack() as ctx:
        cpool = ctx.enter_context(tc.tile_pool(name="coef", bufs=1))
        ppool = ctx.enter_context(tc.tile_pool(name="psum", bufs=1, space="PSUM"))
        xpool = ctx.enter_context(tc.tile_pool(name="x", bufs=4))
        upool = ctx.enter_context(tc.tile_pool(name="tu", bufs=4))
        vpool = ctx.enter_context(tc.tile_pool(name="tv", bufs=4))
        wpool = ctx.enter_context(tc.tile_pool(name="tw", bufs=3))
        opool = ctx.enter_context(tc.tile_pool(name="o", bufs=3))

        for rep in range(reps):
            # ---- coefficient prep ----
            lgT = cpool.tile([16, G], f32, name=f"lgT{rep}")
            nc.scalar.dma_start(out=lgT[:, :], in_=glT[:, :])
            wt = cpool.tile([16, P], f32, name=f"wt{rep}")
            nc.scalar.dma_start(out=wt[:, :], in_=wsel[:, :])
            ones = cpool.tile([1, P], f32, name=f"ones{rep}")
            nc.vector.memset(ones[:, :], 1.0)

            R = {nm: ppool.tile([P, G], f32, name=f"R_{nm}{rep}")
                 for nm in ("cab", "cb", "ca", "c0")}

            # warm the PE while the coefficient chain runs
            nc.tensor.matmul(R["cab"][:, 0:P], ones[:, :], ones[:, :],
                             start=True, stop=True)

            ET = cpool.tile([16, G], f32, name=f"ET{rep}")
            nc.scalar.activation(ET[:, :], lgT[:, :], Act.Exp)

            # numerator rows: den@p0, N0@p32, NA@p64, NB@p96 of R_c0's banks
            # (freed by broadcasting c0 last); NAB@p0 of R_cab's banks.
            num = R["c0"][:, :]
            nab = R["cab"][0:1, :]
            for j in range(0, G, 512):
                nc.tensor.matmul(num[:, j:j + 512], wt[:, :], ET[:, j:j + 512],
                                 start=True, stop=True)
                nc.tensor.matmul(nab[:, j:j + 512], wt[:, 1:2], ET[:, j:j + 512],
                                 start=True, stop=True)

            # den in [0.5, 16*e^5] -> approx reciprocal (2 ULP) is plenty and
            # ~5x faster than the iterative divide on a [1, G] row
            rden = cpool.tile([1, G], f32, name=f"rden{rep}")
            rscr = cpool.tile([1, G], f32, name=f"rscr{rep}")
            nc.vector.reciprocal_approx_accurate(rden[:, :], num[0:1, :],
                                                 rscr[:, :])

            # rows (order matters: cab first; c0 last frees the num strip)
            for nm, src in (("cab", nab), ("cb", num[96:97, :]),
                            ("ca", num[64:65, :]), ("c0", num[32:33, :])):
                row = cpool.tile([1, G], f32, name=f"row_{nm}{rep}")
                nc.vector.tensor_tensor(row[:, :], src, rden[:, :], Op.mult)
                for j in range(0, G, 512):
                    nc.tensor.matmul(R[nm][:, j:j + 512], ones[:, :],
                                     row[:, j:j + 512], start=True, stop=True)

            def bc(r, m):
                return r[:, :].unsqueeze(1).broadcast_to([P, m, G])

            # ---- main loop; group sizes in batch tiles (first/last small) --
            sizes = [1, 1] + [mega] * ((BATCH // P - 4) // mega) + [1, 1]
            assert sum(sizes) == BATCH // P
            rows_lo = 0
            for gi, m in enumerate(sizes):
                rp = P * m
                xin = xs[rows_lo:rows_lo + rp, :].rearrange("(m p) c -> p m c", m=m)
                xt = xpool.tile([P, m, G + 1], f32, name=f"xt{rep}_{gi}",
                                tag="xt")
                nc.sync.dma_start(out=xt[:, :, :], in_=xin)
                a = xt[:, :, 0:G]

                u = upool.tile([P, m, G], f32, name=f"u{rep}_{gi}", tag="u")
                v = vpool.tile([P, m, G], f32, name=f"v{rep}_{gi}", tag="v")
                nc.vector.tensor_tensor(u[:, :, :], a, bc(R["cab"], m), Op.mult)
                nc.vector.tensor_tensor(u[:, :, :], u[:, :, :], bc(R["cb"], m), Op.add)
                nc.vector.tensor_tensor(v[:, :, :], a, bc(R["ca"], m), Op.mult)
                nc.vector.tensor_tensor(v[:, :, :], v[:, :, :], bc(R["c0"], m), Op.add)

                w = wpool.tile([P, m, G], f32, name=f"w{rep}_{gi}", tag="w")
                o = opool.tile([P, m, G], f32, name=f"o{rep}_{gi}", tag="o")
                for sm in range(m):
                    nc.gpsimd.tensor_tensor(w[:, sm, :], u[:, sm, :],
                                            xt[:, sm, 1:G + 1], Op.mult)
                    nc.gpsimd.tensor_tensor(o[:, sm, :], w[:, sm, :],
                                            v[:, sm, :], Op.add)
                    nc.sync.dma_start(
                        out=out[rows_lo + sm * P:rows_lo + (sm + 1) * P, :],
                        in_=o[:, sm, :])
                rows_lo += rp

    nc.compile()
    return nc


def _get_nc(reps=1, mega=MEGA):
    key = (reps, mega)
    if key not in _CACHE:
        _CACHE[key] = _build_nc(reps, mega)
    return _CACHE[key]


def _shard_inputs(x, gate_logits):
    x = np.ascontiguousarray(x, dtype=np.float32)
    gate_logits = np.ascontiguousarray(gate_logits, dtype=np.float32)
    xs_full = np.concatenate([x, x[:, :1]], axis=1)  # wraparound halo
    in_maps = []
    for c in range(N_CORES):
        in_maps.append({
            "xs": np.ascontiguousarray(xs_full[:, c * G:c * G + G + 1]),
            "glT": np.ascontiguousarray(gate_logits[c * G:(c + 1) * G].T),
            "wsel": _WSEL,
        })
    return in_maps


def kernel(x, gate_logits):
    from concourse.bass_utils import run_bass_kernel_spmd

    nc = _get_nc()
    in_maps = _shard_inputs(x, gate_logits)
    res = run_bass_kernel_spmd(nc, in_maps, core_ids=list(range(N_CORES)))
    return np.concatenate([res.results[c]["out"] for c in range(N_CORES)], axis=1)


# revision 22
# speedup vs baseline: 1.1019x; 1.0622x over previous
"""DifferentiableLogicLayer Trainium2 kernel.

Math: reference computes, per batch row t and gate g (G = INPUT_SIZE = 8192):
    a = x[t, g], b = x[t, (g+1) % 8192]            (x uniform in [0,1] -> clip no-op)
    out[t, g] = sum_o softmax(gate_logits[g])_o * op_o(a, b)
Each of the 16 soft ops is linear in {1, a, b, ab}, so with probs p:
    out = C0 + CA*a + CB*b + CAB*a*b
    C0  = p8+..+p15
    CA  = p2+p3+p6+p7-p8-p9-p12-p13
    CB  = p4+p5+p6+p7-p8-p9-p10-p11
    CAB = p1-p2-p4-2*p6-p7+p8+2*p9+p11+p13-p14
Factored: out = ((CAB*a + CB)*b) + (CA*a + C0)  -> 6 elementwise passes.

Sharding: gates across the 8 cores (1024 each; gates are independent, each
needs x columns [g, g+1]).  Per-core inputs:
    xs [2048, 1025] = x cols [1024c .. 1024c+1024] (halo col, wraparound)
    gl [1024, 16]   = gate_logits rows for this core's gates

Coefficient prep runs in a [128 partitions, 8 gates x 16 ops] layout (exp on
ScalarE, subset reductions + combines on VectorE, all on 8-element frees so
they cost ~0.1us each), then each [128, 8] coefficient is reshaped to a
[1, 1024] row by a small SBUF->SBUF DMA and broadcast to a [128, G] PSUM tile
with K=1 matmuls (ones x row).  CAB/CB are finalized first so the main loop
starts as early as possible.

Engine assignment (measured port-sharing rule: GPSIMD's SBUF port is
VectorE's rd1, so GP only contends with DVE instructions whose BOTH tensor
operands live in SBUF — and DVE/GP running 2-port-DVE + GP concurrently is
net-negative):
    VectorE: u = a*R_cab, u += R_cb, v = a*R_ca, v += R_c0   (rd0 + PSUM)
    GPSIMD:  w = u*b, o = w + v                              (pure SBUF)
VectorE runs MEGA=2 batch tiles per instruction (3D APs + step-0 broadcast on
the coefficient operand) to amortize fixed costs; GPSIMD keeps flat 2D
per-subtile APs (3D APs are ~20% slower on the Q7s).
"""

import numpy as np

NUM_GATES = 8192
INPUT_SIZE = 8192
BATCH = 2048
N_CORES = 8
G = NUM_GATES // N_CORES  # 1024 local gates
P = 128
MEGA = 2

_CACHE = {}


def _build_nc(reps=1, mega=MEGA, warm=False, rows_on_act=False, substore=False, bulk_on_act=False):
    from contextlib import ExitStack

    import concourse.bacc as bacc
    import concourse.mybir as mybir
    from concourse.mybir import AluOpType as Op
    from concourse.tile import TileContext

    f32 = mybir.dt.float32
    Ax = mybir.AxisListType
    Act = mybir.ActivationFunctionType

    nc = bacc.Bacc("TRN2", target_bir_lowering=False, debug=False,
                   num_devices=N_CORES)
    xs = nc.dram_tensor("xs", [BATCH, G + 1], f32, kind="ExternalInput").ap()
    gl = nc.dram_tensor("gl", [G, 16], f32, kind="ExternalInput").ap()
    out = nc.dram_tensor("out", [BATCH, G], f32, kind="ExternalOutput").ap()

    with TileContext(nc) as tc, ExitStack() as ctx:
        cpool = ctx.enter_context(tc.tile_pool(name="coef", bufs=1))
        rpool = ctx.enter_context(tc.tile_pool(name="rows", bufs=1))
        ppool = ctx.enter_context(tc.tile_pool(name="psum", bufs=1, space="PSUM"))
        xpool = ctx.enter_context(tc.tile_pool(name="x", bufs=4))
        upool = ctx.enter_context(tc.tile_pool(name="tu", bufs=4))
        vpool = ctx.enter_context(tc.tile_pool(name="tv", bufs=4))
        wpool = ctx.enter_context(tc.tile_pool(name="tw", bufs=3))
        opool = ctx.enter_context(tc.tile_pool(name="o", bufs=3))

        row_dma = nc.scalar.dma_start if rows_on_act else nc.sync.dma_start
        bulk_dma = nc.scalar.dma_start if bulk_on_act else nc.sync.dma_start

        for rep in range(reps):
            # ---- coefficients in [128 partitions, 8 gates x 16 ops] ----
            lg = cpool.tile([P, 8 * 16], f32, name=f"lg{rep}")
            row_dma(out=lg[:, :], in_=gl.rearrange("(p n) o -> p (n o)", p=P))
            E = cpool.tile([P, 8 * 16], f32, name=f"E{rep}")
            nc.scalar.activation(E[:, :], lg[:, :], Act.Exp)
            E3 = E[:, :].rearrange("p (n o) -> p n o", o=16)

            def red(sl, name):
                t = cpool.tile([P, 8], f32, name=name)
                nc.vector.tensor_reduce(t[:, :], sl, Ax.X, Op.add)
                return t

            def Eo(o):
                return E3[:, :, o]

            den = red(E3[:, :, 0:16], f"den{rep}")
            rden = cpool.tile([P, 8], f32, name=f"rden{rep}")
            nc.vector.reciprocal(rden[:, :], den[:, :])

            ones = rpool.tile([1, P], f32, name=f"ones{rep}")
            nc.vector.memset(ones[:, :], 1.0)

            R = {nm: ppool.tile([P, G], f32, name=f"R_{nm}{rep}")
                 for nm in ("cab", "cb", "ca", "c0")}
            if warm:
                nc.tensor.matmul(R["c0"][:, 0:P], ones[:, :], ones[:, :],
                                 start=True, stop=True)

            def finalize(nm, numer):
                c = cpool.tile([P, 8], f32, name=f"c_{nm}{rep}")
                nc.vector.tensor_tensor(c[:, :], numer[:, :], rden[:, :], Op.mult)
                row = rpool.tile([1, G], f32, name=f"row_{nm}{rep}")
                row_dma(out=row[:, :], in_=c[:, :])
                for j in range(0, G, 512):
                    nc.tensor.matmul(R[nm][:, j:j + 512], ones[:, :],
                                     row[:, j:j + 512], start=True, stop=True)

            # CAB = p1-p2-p4-2*p6-p7+p8+2*p9+p11+p13-p14  (needed first)
            nab = cpool.tile([P, 8], f32, name=f"nab{rep}")
            nc.vector.scalar_tensor_tensor(nab[:, :], Eo(6), -2.0, Eo(1), Op.mult, Op.add)
            t2 = cpool.tile([P, 8], f32, name=f"t2{rep}")
            nc.vector.scalar_tensor_tensor(t2[:, :], Eo(9), 2.0, Eo(8), Op.mult, Op.add)
            nc.vector.tensor_tensor(nab[:, :], nab[:, :], t2[:, :], Op.add)
            nc.vector.tensor_tensor(t2[:, :], Eo(11), Eo(13), Op.add)
            nc.vector.tensor_tensor(nab[:, :], nab[:, :], t2[:, :], Op.add)
            nc.vector.tensor_tensor(t2[:, :], Eo(2), Eo(4), Op.add)
            nc.vector.tensor_tensor(t2[:, :], t2[:, :], Eo(7), Op.add)
            nc.vector.tensor_tensor(t2[:, :], t2[:, :], Eo(14), Op.add)
            nc.vector.tensor_tensor(nab[:, :], nab[:, :], t2[:, :], Op.subtract)
            finalize("cab", nab)

            # CB = p4+p5+p6+p7-p8-p9-p10-p11 (second: completes u-chain inputs)
            pb1 = red(E3[:, :, 4:8], f"pb1{rep}")
            pb2 = red(E3[:, :, 8:12], f"pb2{rep}")
            nb = cpool.tile([P, 8], f32, name=f"nb{rep}")
            nc.vector.tensor_tensor(nb[:, :], pb1[:, :], pb2[:, :], Op.subtract)
            finalize("cb", nb)

            # CA = p2+p3+p6+p7-p8-p9-p12-p13
            pa1 = red(E3[:, :, 2:4], f"pa1{rep}")
            pa2 = red(E3[:, :, 6:8], f"pa2{rep}")
            pa3 = red(E3[:, :, 8:10], f"pa3{rep}")
            pa4 = red(E3[:, :, 12:14], f"pa4{rep}")
            na = cpool.tile([P, 8], f32, name=f"na{rep}")
            nc.vector.tensor_tensor(na[:, :], pa1[:, :], pa2[:, :], Op.add)
            nc.vector.tensor_tensor(na[:, :], na[:, :], pa3[:, :], Op.subtract)
            nc.vector.tensor_tensor(na[:, :], na[:, :], pa4[:, :], Op.subtract)
            finalize("ca", na)

            # C0 = p8+..+p15
            n0 = red(E3[:, :, 8:16], f"n0{rep}")
            finalize("c0", n0)

            def bc(r, m):
                return r[:, :].unsqueeze(1).broadcast_to([P, m, G])

            # ---- main loop ----
            ngrp = BATCH // (P * mega)
            for gi in range(ngrp):
                m = mega
                rows_lo = gi * P * m
                xin = xs[rows_lo:rows_lo + P * m, :].rearrange(
                    "(m p) c -> p m c", m=m)
                xt = xpool.tile([P, m, G + 1], f32, name=f"xt{rep}_{gi}", tag="xt")
                bulk_dma(out=xt[:, :, :], in_=xin)
                a = xt[:, :, 0:G]

                u = upool.tile([P, m, G], f32, name=f"u{rep}_{gi}", tag="u")
                v = vpool.tile([P, m, G], f32, name=f"v{rep}_{gi}", tag="v")
                nc.vector.tensor_tensor(u[:, :, :], a, bc(R["cab"], m), Op.mult)
                nc.vector.tensor_tensor(u[:, :, :], u[:, :, :], bc(R["cb"], m), Op.add)
                nc.vector.tensor_tensor(v[:, :, :], a, bc(R["ca"], m), Op.mult)
                nc.vector.tensor_tensor(v[:, :, :], v[:, :, :], bc(R["c0"], m), Op.add)

                w = wpool.tile([P, m, G], f32, name=f"w{rep}_{gi}", tag="w")
                o = opool.tile([P, m, G], f32, name=f"o{rep}_{gi}", tag="o")
                for sm in range(m):
                    nc.gpsimd.tensor_tensor(w[:, sm, :], u[:, sm, :],
                                            xt[:, sm, 1:G + 1], Op.mult)
                    nc.gpsimd.tensor_tensor(o[:, sm, :], w[:, sm, :],
                                            v[:, sm, :], Op.add)
                    if substore:
                        nc.sync.dma_start(
                            out=out[rows_lo + sm * P:rows_lo + (sm + 1) * P, :],
                            in_=o[:, sm, :])
                if not substore:
                    oout = out[rows_lo:rows_lo + P * m, :].rearrange(
                        "(m p) c -> p m c", m=m)
                    nc.sync.dma_start(out=oout, in_=o[:, :, :])

    nc.compile()
    return nc


def _get_nc(reps=1, **kw):
    key = (reps, tuple(sorted(kw.items())))
    if key not in _CACHE:
        _CACHE[key] = _build_nc(reps, **kw)
    return _CACHE[key]


def _shard_inputs(x, gate_logits):
    x = np.ascontiguousarray(x, dtype=np.float32)
    gate_logits = np.ascontiguousarray(gate_logits, dtype=np.float32)
    xs_full = np.concatenate([x, x[:, :1]], axis=1)  # wraparound halo
    in_maps = []
    for c in range(N_CORES):
        in_maps.append({
            "xs": np.ascontiguousarray(xs_full[:, c * G:c * G + G + 1]),
            "gl": np.ascontiguousarray(gate_logits[c * G:(c + 1) * G]),
        })
    return in_maps


def kernel(x, gate_logits):
    from concourse.bass_utils import run_bass_kernel_spmd

    nc = _get_nc()
    in_maps = _shard_inputs(x, gate_logits)
    res = run_bass_kernel_spmd(nc, in_maps, core_ids=list(range(N_CORES)))
    return np.concatenate([res.results[c]["out"] for c in range(N_CORES)], axis=1)
